# revision 10
# baseline (speedup 1.0000x reference)
"""GATv2 3-layer GNN (nn_GCN_10917806866525) on 8 TRN2 NeuronCores.

Sharding: nodes split 12500/core (edge-cut by dst). Per layer, per core:
  A. node-transform GEMM for the local shard (merged [Wl|Wr], act-stationary
     matmul, bf16) -> xl shard + local xr table
  B. AllGather of the xl shard -> full xl table [100352, 64] f32
  C. edge phase over 4096-token windows (host-precomputed so every window has
     unique dst -> dma_scatter_add duplicate-safety), 2 dma_gathers
     (xl[src], xr[dst]) + LeakyReLU/att-dot/exp + scatter of [w*xl | w] into
     R rotating DRAM accumulators
  D. merge accs, divide by the w-column (softmax denominator; max-subtraction
     skipped - exponents are small in fp32), + (bl+bias), exact BatchNorm via
     ones-matmul partition reduce + AllReduce of [2,H] stats, ReLU
  E. PE-transpose of h for the next layer's GEMM
Pooling: per-node-tile indicator matmul accumulated in PSUM -> [64,17]
partials per core; host sums cores, divides counts, applies final linear.
"""

import time

import numpy as np
import ml_dtypes

import concourse.bacc as bacc
import concourse.bass as bass
import concourse.mybir as mybir
from concourse import tile
from concourse.bass_utils import run_bass_kernel_spmd

FP32 = mybir.dt.float32
BF16 = mybir.dt.bfloat16
I16 = mybir.dt.int16

NCORES = 8
N = 100000
F = 128
E = 1600000
G = 64
EPS = 1e-5
NEG_SLOPE = 0.2

SHARD = N // NCORES          # 12500
PADSH = 12544                # 98 * 128
NT = PADSH // 128            # 98 node tiles per shard
TABN = NCORES * PADSH        # 100352 rows in the all-gathered xl table
BLOCK = 32768                # int16 gather block
NBLK = (TABN + BLOCK - 1) // BLOCK   # 4
WCAP = 4096                  # tokens per window (= one gather/scatter op)
WT = WCAP // 128             # 32 token columns per window
RACC = 4                     # rotating scatter accumulators
N0 = float(N)                # true node count for BN stats

LAYERS = [(128, 64), (64, 32), (32, 16)]  # (F_in, H)
TW = 64                      # gather table width (f32, 256B rows)
ACCW = 128                   # accumulator row stride (512B)


# ---------------------------------------------------------------- host prep

def _wrap16(idx):
    """[n] int -> [16, n/16] int16 wrapped (idx j at [j%16, j//16])."""
    return np.ascontiguousarray(np.asarray(idx, np.int16).reshape(-1, 16).T)


def _build_windows(edge_index):
    """Partition edges by dst core; per core, bucket by src table block and
    pack into 4096-token windows with per-window-unique dst_local. Dummy
    tokens (src row 0 of the bucket, dst pad row 12500) fill windows so all
    cores share one SPMD-identical window layout.

    Returns (srcidx[c], dstidx[c]) wrapped [16, TOT/16] int16 arrays and
    win_bucket: list of bucket id per window."""
    src = np.concatenate([edge_index[0], np.arange(N, dtype=np.int64)])
    dst = np.concatenate([edge_index[1], np.arange(N, dtype=np.int64)])
    core = dst // SHARD
    dst_local = (dst - core * SHARD).astype(np.int64)
    src_tab = (src // SHARD) * PADSH + (src % SHARD)
    bucket = src_tab // BLOCK
    src_blk = (src_tab - bucket * BLOCK).astype(np.int64)

    # per (core, bucket) edge lists
    per = [[None] * NBLK for _ in range(NCORES)]
    for c in range(NCORES):
        mc = core == c
        for b in range(NBLK):
            m = mc & (bucket == b)
            per[c][b] = (src_blk[m], dst_local[m])

    # global window count per bucket
    wb = []
    for b in range(NBLK):
        need = 0
        for c in range(NCORES):
            s, d = per[c][b]
            cnt = np.bincount(d, minlength=SHARD)
            need = max(need, int(np.ceil(len(d) / (WCAP - 96))) + 1, int(cnt.max()))
        wb.append(need)

    srcidx = [[] for _ in range(NCORES)]
    dstidx = [[] for _ in range(NCORES)]
    win_bucket = []
    for b in range(NBLK):
        W = wb[b]
        win_bucket += [b] * W
        for c in range(NCORES):
            s, d = per[c][b]
            order = np.argsort(d, kind="stable")
            s, d = s[order], d[order]
            # occurrence index within (dst)
            occ = np.arange(len(d)) - np.searchsorted(d, d, side="left")
            w_of = (d + occ) % W
            wins_s = [[] for _ in range(W)]
            wins_d = [[] for _ in range(W)]
            wdst = [set() for _ in range(W)]
            spill_s, spill_d = [], []
            for i in range(len(d)):
                w = w_of[i]
                if len(wins_s[w]) < WCAP and d[i] not in wdst[w]:
                    wins_s[w].append(s[i])
                    wins_d[w].append(d[i])
                    wdst[w].add(d[i])
                else:
                    spill_s.append(s[i])
                    spill_d.append(d[i])
            for si, di in zip(spill_s, spill_d):
                for w in range(W):
                    if len(wins_s[w]) < WCAP and di not in wdst[w]:
                        wins_s[w].append(si)
                        wins_d[w].append(di)
                        wdst[w].add(di)
                        break
                else:
                    raise RuntimeError("window spill placement failed")
            for w in range(W):
                pad = WCAP - len(wins_s[w])
                arr_s = np.array(wins_s[w] + [0] * pad, np.int64)
                arr_d = np.array(wins_d[w] + [SHARD] * pad, np.int64)
                srcidx[c].append(arr_s)
                dstidx[c].append(arr_d)

    src_w = [np.tile(_wrap16(np.concatenate(srcidx[c])), (8, 1)) for c in range(NCORES)]
    dst_w = [np.tile(_wrap16(np.concatenate(dstidx[c])), (8, 1)) for c in range(NCORES)]
    return src_w, dst_w, win_bucket


# ---------------------------------------------------------------- device nc

def _build_nc(win_bucket, nlayers=3, debug=False):
    TOTW = len(win_bucket)
    TOT16 = TOTW * WCAP // 16
    nc = bacc.Bacc("TRN2", target_bir_lowering=False, debug=False,
                   num_devices=NCORES)

    xT = nc.declare_dram_parameter("xT", [128, PADSH], BF16, isOutput=False)
    sidx_in = nc.declare_dram_parameter("sidx", [128, TOT16], I16, isOutput=False)
    didx_in = nc.declare_dram_parameter("didx", [128, TOT16], I16, isOutput=False)
    ident_in = nc.declare_dram_parameter("ident", [128, 128], BF16, isOutput=False)
    iota_in = nc.declare_dram_parameter("iota", [128, G], FP32, isOutput=False)
    batch_in = nc.declare_dram_parameter("batchf", [128, NT], FP32, isOutput=False)
    padmask_in = nc.declare_dram_parameter("padmask", [128, 1], FP32, isOutput=False)
    wcat_in, attr_in, bout_in, bxr_in, gam_in, bet_in = [], [], [], [], [], []
    for li, (fi, h) in enumerate(LAYERS):
        wcat_in.append(nc.declare_dram_parameter(f"wcat{li}", [fi, 2 * h], BF16, isOutput=False))
        attr_in.append(nc.declare_dram_parameter(f"attr{li}", [128, h], FP32, isOutput=False))
        bout_in.append(nc.declare_dram_parameter(f"bout{li}", [128, h], FP32, isOutput=False))
        bxr_in.append(nc.declare_dram_parameter(f"bxr{li}", [128, 2 * h], FP32, isOutput=False))
        gam_in.append(nc.declare_dram_parameter(f"gam{li}", [1, h], FP32, isOutput=False))
        bet_in.append(nc.declare_dram_parameter(f"bet{li}", [1, h], FP32, isOutput=False))
    pool_out = nc.declare_dram_parameter("pool", [G, 17], FP32, isOutput=True)
    if debug:
        xl_dump = nc.declare_dram_parameter("xl_dump", [PADSH, TW], FP32, isOutput=True)
        xr_dump = nc.declare_dram_parameter("xr_dump", [PADSH, TW], FP32, isOutput=True)
        accm_dump = nc.declare_dram_parameter("accm_dump", [PADSH, ACCW], FP32, isOutput=True)
        stpre_dump = nc.declare_dram_parameter("stpre_dump", [1, 128], FP32, isOutput=True)
        stpost_dump = nc.declare_dram_parameter("stpost_dump", [1, 128], FP32, isOutput=True)
        out_dump = nc.declare_dram_parameter("out_dump", [128, NT * TW], FP32, isOutput=True)

    xl_bounce = nc.dram_tensor("xl_bounce", [PADSH, TW], FP32)
    xl_full = nc.dram_tensor("xl_full", [TABN, TW], FP32, addr_space="Shared")
    xr_tab = nc.dram_tensor("xr_tab", [PADSH, TW], FP32)
    accs = [nc.dram_tensor(f"acc{r}", [PADSH, ACCW], FP32) for r in range(RACC)]
    bn_in = nc.dram_tensor("bn_in", [1, 128], FP32)
    bn_out = nc.dram_tensor("bn_out", [1, 128], FP32, addr_space="Shared")

    # block row counts in the xl table
    blk_rows = [min(BLOCK, TABN - b * BLOCK) for b in range(NBLK)]

    with tile.TileContext(nc) as tc:
        with (
            tc.tile_pool(name="persist", bufs=1) as pp,
            tc.tile_pool(name="gemm", bufs=4) as gp,
            tc.tile_pool(name="gpsum", bufs=2, space="PSUM") as gpp,
            tc.tile_pool(name="spsum", bufs=1, space="PSUM") as spp,
            tc.tile_pool(name="win", bufs=3) as wp,
            tc.tile_pool(name="ep", bufs=3) as ep,
            tc.tile_pool(name="misc", bufs=2) as mp,
        ):
            # ---- persistent loads
            xT_sb = pp.tile([128, PADSH], BF16)
            nc.sync.dma_start(out=xT_sb[:], in_=xT[:])
            ident_sb = pp.tile([128, 128], BF16)
            nc.sync.dma_start(out=ident_sb[:], in_=ident_in[:])
            iota_sb = pp.tile([128, G], FP32)
            nc.sync.dma_start(out=iota_sb[:], in_=iota_in[:])
            batch_sb = pp.tile([128, NT], FP32)
            nc.sync.dma_start(out=batch_sb[:], in_=batch_in[:])
            padmask_sb = pp.tile([128, 1], FP32)
            nc.sync.dma_start(out=padmask_sb[:], in_=padmask_in[:])
            ones_sb = pp.tile([128, 1], FP32)
            nc.vector.memset(ones_sb[:], 1.0)
            ones_row = pp.tile([1, 128], FP32)
            nc.vector.memset(ones_row[:], 1.0)
            zero_sb = pp.tile([128, 1024], FP32)
            nc.vector.memset(zero_sb[:], 0.0)
            z3 = zero_sb[:].rearrange("p (a b) -> p a b", b=128)

            hT_sb = pp.tile([128, PADSH], BF16)   # transposed h for next layer
            out_sb = pp.tile([128, NT * TW], FP32)  # pre-BN aggregates
            hpool_sb = pp.tile([128, NT * 17], FP32)  # L3 pooling payload

            for li, (fi, h) in enumerate(LAYERS[:nlayers]):
                h2 = 2 * h
                EL = h + 1  # scatter element: [w*xl | w]
                # ---- layer consts
                wcat_sb = mp.tile([fi, h2], BF16, tag="wcat")
                nc.sync.dma_start(out=wcat_sb[:], in_=wcat_in[li][:])
                attr_sb = mp.tile([128, h], FP32, tag="attr")
                nc.sync.dma_start(out=attr_sb[:], in_=attr_in[li][:])
                bout_sb = mp.tile([128, h], FP32, tag="bout")
                nc.sync.dma_start(out=bout_sb[:], in_=bout_in[li][:])
                bxr_sb = mp.tile([128, h2], FP32, tag="bxr")
                nc.sync.dma_start(out=bxr_sb[:], in_=bxr_in[li][:])
                gam_sb = mp.tile([1, h], FP32, tag="gam")
                nc.sync.dma_start(out=gam_sb[:], in_=gam_in[li][:])
                bet_sb = mp.tile([1, h], FP32, tag="bet")
                nc.sync.dma_start(out=bet_sb[:], in_=bet_in[li][:])

                # ---- A: GEMM -> xl_bounce + xr_tab
                for t in range(NT):
                    if li == 0:
                        lhsT = xT_sb[:, t * 128:(t + 1) * 128]
                    else:
                        lhsT = hT_sb[:fi, t * 128:(t + 1) * 128]
                    ps = gpp.tile([128, h2], FP32, tag="gemm_ps")
                    nc.tensor.matmul(ps[:], lhsT, wcat_sb[:], start=True, stop=True)
                    sb = gp.tile([128, h2], FP32, tag="gemm_sb")
                    nc.vector.tensor_tensor(sb[:], ps[:], bxr_sb[:], mybir.AluOpType.add)
                    nc.sync.dma_start(
                        out=xl_bounce[t * 128:(t + 1) * 128, 0:h], in_=sb[:, 0:h])
                    nc.sync.dma_start(
                        out=xr_tab[t * 128:(t + 1) * 128, 0:h], in_=sb[:, h:h2])

                # ---- B: AllGather xl table
                nc.gpsimd.collective_compute(
                    "AllGather", mybir.AluOpType.bypass,
                    replica_groups=[list(range(NCORES))],
                    ins=[xl_bounce[:].opt()], outs=[xl_full[:].opt()],
                )

                if debug and li == 0:
                    for t in range(NT):
                        dt_ = gp.tile([128, TW], FP32, tag="dmp")
                        nc.sync.dma_start(out=dt_[:], in_=xl_bounce[t * 128:(t + 1) * 128, :])
                        nc.sync.dma_start(out=xl_dump[t * 128:(t + 1) * 128, :], in_=dt_[:])
                        dt2 = gp.tile([128, TW], FP32, tag="dmp2")
                        nc.sync.dma_start(out=dt2[:], in_=xr_tab[t * 128:(t + 1) * 128, :])
                        nc.sync.dma_start(out=xr_dump[t * 128:(t + 1) * 128, :], in_=dt2[:])

                # ---- C: zero accumulators
                for r in range(RACC):
                    for k in range(PADSH // 896):
                        acc3 = accs[r][k * 896:(k + 1) * 896, :].rearrange(
                            "(p a) b -> p a b", p=128)
                        nc.sync.dma_start(out=acc3, in_=z3[:, 0:7, :])

                # ---- D: edge windows
                for w, b in enumerate(win_bucket):
                    c0 = w * (WCAP // 16)
                    si = wp.tile([128, WCAP // 16], I16, tag="sidx")
                    nc.sync.dma_start(out=si[:], in_=sidx_in[:, c0:c0 + WCAP // 16])
                    di = wp.tile([128, WCAP // 16], I16, tag="didx")
                    nc.sync.dma_start(out=di[:], in_=didx_in[:, c0:c0 + WCAP // 16])
                    xlg = wp.tile([128, WT * TW], FP32, tag="xlg")
                    xl3 = xlg[:].rearrange("p (a b) -> p a b", b=TW)
                    nc.gpsimd.dma_gather(
                        out_ap=xl3,
                        in_ap=xl_full[b * BLOCK:b * BLOCK + blk_rows[b], :],
                        idxs_ap=si[:], num_idxs=WCAP, num_idxs_reg=WCAP,
                        elem_size=TW, single_packet=False,
                    )
                    xrg = wp.tile([128, WT * TW], FP32, tag="xrg")
                    xr3 = xrg[:].rearrange("p (a b) -> p a b", b=TW)
                    nc.gpsimd.dma_gather(
                        out_ap=xr3, in_ap=xr_tab[:],
                        idxs_ap=di[:], num_idxs=WCAP, num_idxs_reg=WCAP,
                        elem_size=TW, single_packet=False,
                    )
                    # s = xl + xr ; lr = lrelu(s) ; t = lr*att (all into xrg)
                    nc.vector.tensor_tensor(
                        xr3[:, :, 0:h], xl3[:, :, 0:h], xr3[:, :, 0:h],
                        mybir.AluOpType.add)
                    sc = wp.tile([128, WT * TW], FP32, tag="sc")
                    s3 = sc[:].rearrange("p (a b) -> p a b", b=TW)
                    nc.scalar.activation(
                        s3[:, :, 0:h], xr3[:, :, 0:h],
                        mybir.ActivationFunctionType.Copy, scale=NEG_SLOPE)
                    nc.vector.tensor_tensor(
                        xr3[:, :, 0:h], xr3[:, :, 0:h], s3[:, :, 0:h],
                        mybir.AluOpType.max)
                    nc.vector.tensor_tensor(
                        xr3[:, :, 0:h], xr3[:, :, 0:h],
                        attr_sb[:].unsqueeze(1).broadcast_to((128, WT, h)),
                        mybir.AluOpType.mult)
                    ew = wp.tile([128, WT], FP32, tag="ew")
                    nc.vector.tensor_reduce(
                        ew[:], xr3[:, :, 0:h], axis=mybir.AxisListType.X,
                        op=mybir.AluOpType.add)
                    nc.scalar.activation(
                        ew[:], ew[:], mybir.ActivationFunctionType.Exp)
                    pay = wp.tile([128, WT * EL], FP32, tag="pay")
                    p3 = pay[:].rearrange("p (a b) -> p a b", b=EL)
                    nc.vector.tensor_tensor(
                        p3[:, :, 0:h], xl3[:, :, 0:h],
                        ew[:].unsqueeze(2).broadcast_to((128, WT, h)),
                        mybir.AluOpType.mult)
                    nc.vector.tensor_copy(p3[:, :, h:EL], ew[:].unsqueeze(2))
                    nc.gpsimd.dma_scatter_add(
                        accs[w % RACC][:, 0:EL], p3, di[:], WCAP, WCAP, EL,
                        elem_step=ACCW, single_packet=False,
                    )

                # ---- E: merge accs, divide, bias, stats
                st2 = ep.tile([128, 2 * h], FP32, tag="st2")
                nc.vector.memset(st2[:], 0.0)
                ssum = st2[:, 0:h]
                sqsum = st2[:, h:2 * h]
                for t in range(NT):
                    at = ep.tile([128, ACCW], FP32, tag="at")
                    nc.sync.dma_start(out=at[:], in_=accs[0][t * 128:(t + 1) * 128, :])
                    for r in range(1, RACC):
                        ar = ep.tile([128, ACCW], FP32, tag="ar")
                        nc.sync.dma_start(out=ar[:], in_=accs[r][t * 128:(t + 1) * 128, :])
                        nc.vector.tensor_tensor(at[:, 0:EL], at[:, 0:EL], ar[:, 0:EL],
                                                mybir.AluOpType.add)
                    # denom -> reciprocal
                    if debug and li == 0:
                        nc.sync.dma_start(out=accm_dump[t * 128:(t + 1) * 128, 0:EL],
                                          in_=at[:, 0:EL])
                    rec = ep.tile([128, 1], FP32, tag="rec")
                    nc.vector.tensor_scalar_add(rec[:], at[:, h:EL], 1e-30)
                    nc.vector.reciprocal(rec[:], rec[:])
                    op = out_sb[:, t * TW:t * TW + h]
                    nc.vector.tensor_scalar_mul(op, at[:, 0:h], rec[:])
                    nc.vector.tensor_tensor(op, op, bout_sb[:], mybir.AluOpType.add)
                    if t == NT - 1:
                        # zero pad nodes 12500..12543 (partitions 84.. of the
                        # last tile) via per-partition mask multiply
                        nc.vector.tensor_scalar_mul(op, op, padmask_sb[:])
                    nc.vector.tensor_tensor(ssum, ssum, op, mybir.AluOpType.add)
                    sq = ep.tile([128, h], FP32, tag="sq")
                    nc.scalar.activation(sq[:], op, mybir.ActivationFunctionType.Square)
                    nc.vector.tensor_tensor(sqsum, sqsum, sq[:], mybir.AluOpType.add)

                # partition-reduce stats via ones matmul -> [1, 2h]
                stp = spp.tile([1, 2 * h], FP32, tag="stats_ps")
                nc.tensor.matmul(stp[:], ones_sb[:, 0:1], st2[:], start=True, stop=True)
                st_sb = ep.tile([1, 128], FP32, tag="st_sb")
                nc.vector.memset(st_sb[:], 0.0)
                nc.scalar.copy(st_sb[0:1, 0:2 * h], stp[:])
                nc.sync.dma_start(out=bn_in[:], in_=st_sb[:])
                if debug and li == 0:
                    nc.sync.dma_start(out=stpre_dump[:], in_=st_sb[:])
                nc.gpsimd.collective_compute(
                    "AllReduce", mybir.AluOpType.add,
                    replica_groups=[list(range(NCORES))],
                    ins=[bn_in[:].opt()], outs=[bn_out[:].opt()],
                )
                stg = ep.tile([1, 128], FP32, tag="stg")
                nc.sync.dma_start(out=stg[:], in_=bn_out[:])
                if debug and li == 0:
                    nc.sync.dma_start(out=stpost_dump[:], in_=stg[:])
                # a = gamma * rsqrt(var+eps); b = beta - mu * a   (on 1 partition)
                mu = ep.tile([1, h], FP32, tag="mu")
                nc.vector.tensor_scalar_mul(mu[:], stg[0:1, 0:h], 1.0 / N0)
                var = ep.tile([1, h], FP32, tag="var")
                nc.vector.tensor_scalar_mul(var[:], stg[0:1, h:2 * h], 1.0 / N0)
                musq = ep.tile([1, h], FP32, tag="musq")
                nc.scalar.activation(musq[:], mu[:], mybir.ActivationFunctionType.Square)
                nc.vector.tensor_tensor(var[:], var[:], musq[:], mybir.AluOpType.subtract)
                nc.vector.tensor_scalar_add(var[:], var[:], EPS)
                nc.scalar.activation(var[:], var[:], mybir.ActivationFunctionType.Sqrt)
                nc.vector.reciprocal(var[:], var[:])   # rsqrt
                arow = ep.tile([1, h], FP32, tag="arow")
                nc.vector.tensor_tensor(arow[:], var[:], gam_sb[:], mybir.AluOpType.mult)
                brow = ep.tile([1, h], FP32, tag="brow")
                nc.vector.tensor_tensor(brow[:], mu[:], arow[:], mybir.AluOpType.mult)
                nc.vector.tensor_tensor(brow[:], bet_sb[:], brow[:], mybir.AluOpType.subtract)
                # broadcast a,b across partitions via K=1 matmul
                abp = spp.tile([128, h], FP32, tag="ab_ps")
                nc.tensor.matmul(abp[:], ones_row[:], arow[:], start=True, stop=True)
                a_bc = ep.tile([128, h], FP32, tag="a_bc")
                nc.scalar.copy(a_bc[:], abp[:])
                abp2 = spp.tile([128, h], FP32, tag="ab_ps")
                nc.tensor.matmul(abp2[:], ones_row[:], brow[:], start=True, stop=True)
                b_bc = ep.tile([128, h], FP32, tag="b_bc")
                nc.scalar.copy(b_bc[:], abp2[:])

                # ---- normalize + relu (+ transpose or pooling payload)
                if li < 2:
                    for t in range(NT):
                        op = out_sb[:, t * TW:t * TW + h]
                        hb = gp.tile([128, h], BF16, tag="hb")
                        nc.vector.tensor_tensor(op, op, a_bc[:], mybir.AluOpType.mult)
                        nc.vector.tensor_tensor(op, op, b_bc[:], mybir.AluOpType.add)
                        nc.scalar.activation(hb[:], op,
                                             mybir.ActivationFunctionType.Relu)
                        tp = gpp.tile([h, 128], BF16, tag="tr_ps")
                        nc.tensor.transpose(tp[:], hb[:], ident_sb[:])
                        nc.scalar.copy(hT_sb[0:h, t * 128:(t + 1) * 128], tp[:])
                else:
                    for t in range(NT):
                        op = out_sb[:, t * TW:t * TW + h]
                        nc.vector.tensor_tensor(op, op, a_bc[:], mybir.AluOpType.mult)
                        nc.vector.tensor_tensor(op, op, b_bc[:], mybir.AluOpType.add)
                        hp3 = hpool_sb[:, t * 17:(t + 1) * 17]
                        nc.scalar.activation(hp3[0:128, 0:h], op,
                                             mybir.ActivationFunctionType.Relu)
                        nc.vector.memset(hpool_sb[:, t * 17 + 16:t * 17 + 17], 1.0)

            if debug:
                nc.sync.dma_start(out=out_dump[:], in_=out_sb[:])

            # ---- pooling: indicator matmuls accumulate [G, 17]
            if nlayers == 3:
                plp = spp.tile([G, 17], FP32, tag="pool_ps")
                for t in range(NT):
                    ind = gp.tile([128, G], FP32, tag="ind")
                    nc.vector.tensor_tensor(
                        ind[:], batch_sb[:, t:t + 1].broadcast_to((128, G)),
                        iota_sb[:], mybir.AluOpType.is_equal)
                    nc.tensor.matmul(plp[:], ind[:], hpool_sb[:, t * 17:(t + 1) * 17],
                                     start=(t == 0), stop=(t == NT - 1))
                pl_sb = gp.tile([G, 17], FP32, tag="pl_sb")
                nc.scalar.copy(pl_sb[:], plp[:])
                nc.sync.dma_start(out=pool_out[:], in_=pl_sb[:])
            else:
                nc.vector.memset(hpool_sb[:], 0.0)
                pl_sb = gp.tile([G, 17], FP32, tag="pl_sb")
                nc.vector.memset(pl_sb[:], 0.0)
                nc.sync.dma_start(out=pool_out[:], in_=pl_sb[:])

    nc.compile()
    return nc


# ---------------------------------------------------------------- kernel()

LAST_EXEC_NS = None

def _make_in_maps(inputs):
    x = np.asarray(inputs["x"], np.float32)
    ei = np.asarray(inputs["edge_index"], np.int64)
    batch = np.asarray(inputs["batch"], np.int64)
    src_w, dst_w, win_bucket = _build_windows(ei)
    # xT shards [128, PADSH] bf16
    xT = np.zeros((NCORES, 128, PADSH), ml_dtypes.bfloat16)
    xt_full = np.ascontiguousarray(x.T).astype(ml_dtypes.bfloat16)  # [128, N]
    for c in range(NCORES):
        xT[c, :, :SHARD] = xt_full[:, c * SHARD:(c + 1) * SHARD]

    ident = np.eye(128, dtype=ml_dtypes.bfloat16)
    iota = np.tile(np.arange(G, dtype=np.float32)[None, :], (128, 1))
    batchf = np.full((NCORES, 128, NT), -1.0, np.float32)
    for c in range(NCORES):
        loc = batch[c * SHARD:(c + 1) * SHARD].astype(np.float32)
        pad = np.full(PADSH - SHARD, -1.0, np.float32)
        batchf[c] = np.concatenate([loc, pad]).reshape(NT, 128).T

    const_maps = {"ident": ident, "iota": iota}
    for li, (fi, h) in enumerate(LAYERS):
        Wl = np.asarray(inputs[f"Wl{li+1}"], np.float32)
        bl = np.asarray(inputs[f"bl{li+1}"], np.float32)
        Wr = np.asarray(inputs[f"Wr{li+1}"], np.float32)
        br = np.asarray(inputs[f"br{li+1}"], np.float32)
        att = np.asarray(inputs[f"att{li+1}"], np.float32)
        bias = np.asarray(inputs[f"bias{li+1}"], np.float32)
        gam = np.asarray(inputs[f"gamma{li+1}"], np.float32)
        bet = np.asarray(inputs[f"beta{li+1}"], np.float32)
        const_maps[f"wcat{li}"] = np.concatenate([Wl, Wr], 1).astype(ml_dtypes.bfloat16)
        const_maps[f"attr{li}"] = np.tile(att[None, :], (128, 1)).astype(np.float32)
        const_maps[f"bout{li}"] = np.tile((bl + bias)[None, :], (128, 1)).astype(np.float32)
        bxr = np.concatenate([np.zeros(h, np.float32), bl + br])
        const_maps[f"bxr{li}"] = np.tile(bxr[None, :], (128, 1)).astype(np.float32)
        const_maps[f"gam{li}"] = gam[None, :].astype(np.float32)
        const_maps[f"bet{li}"] = bet[None, :].astype(np.float32)

    padmask = (np.arange(128) + (NT - 1) * 128 < SHARD).astype(np.float32)[:, None]
    in_maps = [
        {"xT": xT[c], "sidx": src_w[c], "didx": dst_w[c], "batchf": batchf[c],
         "padmask": padmask, **const_maps}
        for c in range(NCORES)
    ]
    return in_maps, win_bucket


def kernel(**inputs):
    in_maps, win_bucket = _make_in_maps(inputs)
    nc = _build_nc(win_bucket)

    global LAST_EXEC_NS
    t0 = time.time()
    res = run_bass_kernel_spmd(nc, in_maps, core_ids=list(range(NCORES)))
    LAST_EXEC_NS = (time.time() - t0) * 1e9

    pool = np.zeros((G, 17), np.float64)
    for c in range(NCORES):
        pool += np.asarray(res.results[c]["pool"], np.float64)
    sums, cnt = pool[:, :16], pool[:, 16]
    pooled = sums / np.maximum(cnt, 1.0)[:, None]
    linW = np.asarray(inputs["linW"], np.float32)
    linb = np.asarray(inputs["linb"], np.float32)
    return (pooled.astype(np.float32) @ linW + linb).astype(np.float32)


# revision 12
# speedup vs baseline: 1.5177x; 1.5177x over previous
"""GATv2 3-layer GNN (nn_GCN_10917806866525) on 8 TRN2 NeuronCores.

Sharding: nodes split 12500/core (edge-cut by dst). Per layer, per core:
  A. node-transform GEMM for the local shard (merged [Wl|Wr], act-stationary
     matmul, bf16) -> xl shard + local xr table
  B. AllGather of the xl shard -> full xl table [100352, 64] f32
  C. edge phase over 4096-token windows (host-precomputed so every window has
     unique dst -> dma_scatter_add duplicate-safety), 2 dma_gathers
     (xl[src], xr[dst]) + LeakyReLU/att-dot/exp + scatter of [w*xl | w] into
     R rotating DRAM accumulators
  D. merge accs, divide by the w-column (softmax denominator; max-subtraction
     skipped - exponents are small in fp32), + (bl+bias), exact BatchNorm via
     ones-matmul partition reduce + AllReduce of [2,H] stats, ReLU
  E. PE-transpose of h for the next layer's GEMM
Pooling: per-node-tile indicator matmul accumulated in PSUM -> [64,17]
partials per core; host sums cores, divides counts, applies final linear.
"""

import time

import numpy as np
import ml_dtypes

import concourse.bacc as bacc
import concourse.bass as bass
import concourse.mybir as mybir
from concourse import tile
from concourse.bass_utils import run_bass_kernel_spmd

FP32 = mybir.dt.float32
BF16 = mybir.dt.bfloat16
I16 = mybir.dt.int16

NCORES = 8
N = 100000
F = 128
E = 1600000
G = 64
EPS = 1e-5
NEG_SLOPE = 0.2

SHARD = N // NCORES          # 12500
PADSH = 12544                # 98 * 128
NT = PADSH // 128            # 98 node tiles per shard
TABN = NCORES * PADSH        # 100352 rows in the all-gathered xl table
BLOCK = 32768                # int16 gather block
NBLK = (TABN + BLOCK - 1) // BLOCK   # 4
WCAP = 4096                  # tokens per window (= one gather/scatter op)
WT = WCAP // 128             # 32 token columns per window
RACC = 4                     # rotating scatter accumulators
N0 = float(N)                # true node count for BN stats

LAYERS = [(128, 64), (64, 32), (32, 16)]  # (F_in, H)
TW = 64                      # gather table width (f32, 256B rows)
ACCW = 128                   # accumulator row stride (512B)


# ---------------------------------------------------------------- host prep

def _wrap16(idx):
    """[n] int -> [16, n/16] int16 wrapped (idx j at [j%16, j//16])."""
    return np.ascontiguousarray(np.asarray(idx, np.int16).reshape(-1, 16).T)


def _build_windows(edge_index):
    """Partition edges by dst core; per core, bucket by src table block and
    pack into 4096-token windows with per-window-unique dst_local. Dummy
    tokens (src row 0 of the bucket, dst pad row 12500) fill windows so all
    cores share one SPMD-identical window layout.

    Returns (srcidx[c], dstidx[c]) wrapped [16, TOT/16] int16 arrays and
    win_bucket: list of bucket id per window."""
    src = np.concatenate([edge_index[0], np.arange(N, dtype=np.int64)])
    dst = np.concatenate([edge_index[1], np.arange(N, dtype=np.int64)])
    core = dst // SHARD
    dst_local = (dst - core * SHARD).astype(np.int64)
    src_tab = (src // SHARD) * PADSH + (src % SHARD)
    bucket = src_tab // BLOCK
    src_blk = (src_tab - bucket * BLOCK).astype(np.int64)

    # per (core, bucket) edge lists
    per = [[None] * NBLK for _ in range(NCORES)]
    for c in range(NCORES):
        mc = core == c
        for b in range(NBLK):
            m = mc & (bucket == b)
            per[c][b] = (src_blk[m], dst_local[m])

    # global window count per bucket
    wb = []
    for b in range(NBLK):
        need = 0
        for c in range(NCORES):
            s, d = per[c][b]
            cnt = np.bincount(d, minlength=SHARD)
            need = max(need, int(np.ceil(len(d) / (WCAP - 96))) + 1, int(cnt.max()))
        wb.append(need)

    srcidx = [[] for _ in range(NCORES)]
    dstidx = [[] for _ in range(NCORES)]
    win_bucket = []
    for b in range(NBLK):
        W = wb[b]
        win_bucket += [b] * W
        for c in range(NCORES):
            s, d = per[c][b]
            order = np.argsort(d, kind="stable")
            s, d = s[order], d[order]
            occ = np.arange(len(d)) - np.searchsorted(d, d, side="left")
            w_of = (d + occ) % W
            counts = np.bincount(w_of, minlength=W)
            if counts.max() > WCAP:
                # rare capacity overflow: move excess edges greedily
                has = np.zeros((W, SHARD + 1), bool)
                has[w_of, d] = True
                for w in np.where(counts > WCAP)[0]:
                    idxs = np.where(w_of == w)[0]
                    for i in idxs[WCAP:]:
                        cand = np.where((counts < WCAP) & ~has[:, d[i]])[0]
                        if len(cand) == 0:
                            raise RuntimeError("window spill placement failed")
                        has[w, d[i]] = False
                        has[cand[0], d[i]] = True
                        counts[w] -= 1
                        counts[cand[0]] += 1
                        w_of[i] = cand[0]
            o2 = np.lexsort((d, w_of))
            s2, d2, w2 = s[o2], d[o2], w_of[o2]
            starts = np.concatenate([[0], np.cumsum(counts)[:-1]])
            pos = np.arange(len(d2)) - np.repeat(starts, counts)
            out_s = np.zeros((W, WCAP), np.int64)
            out_d = np.full((W, WCAP), SHARD, np.int64)
            out_s[w2, pos] = s2
            out_d[w2, pos] = d2
            srcidx[c].append(out_s.reshape(-1))
            dstidx[c].append(out_d.reshape(-1))

    src_w = [_wrap16(np.concatenate(srcidx[c])) for c in range(NCORES)]
    dst_w = [_wrap16(np.concatenate(dstidx[c])) for c in range(NCORES)]
    return src_w, dst_w, win_bucket


# ---------------------------------------------------------------- device nc

def _build_nc(win_bucket, nlayers=3, debug=False):
    TOTW = len(win_bucket)
    TOT16 = TOTW * WCAP // 16
    nc = bacc.Bacc("TRN2", target_bir_lowering=False, debug=False,
                   num_devices=NCORES)

    xT = nc.declare_dram_parameter("xT", [128, PADSH], BF16, isOutput=False)
    sidx_in = nc.declare_dram_parameter("sidx", [16, TOT16], I16, isOutput=False)
    didx_in = nc.declare_dram_parameter("didx", [16, TOT16], I16, isOutput=False)
    sidx_rep = nc.dram_tensor("sidx_rep", [128, TOT16], I16)
    didx_rep = nc.dram_tensor("didx_rep", [128, TOT16], I16)
    ident_in = nc.declare_dram_parameter("ident", [128, 128], BF16, isOutput=False)
    iota_in = nc.declare_dram_parameter("iota", [128, G], FP32, isOutput=False)
    batch_in = nc.declare_dram_parameter("batchf", [128, NT], FP32, isOutput=False)
    padmask_in = nc.declare_dram_parameter("padmask", [128, 1], FP32, isOutput=False)
    wcat_in, attr_in, bout_in, bxr_in, gam_in, bet_in = [], [], [], [], [], []
    for li, (fi, h) in enumerate(LAYERS):
        wcat_in.append(nc.declare_dram_parameter(f"wcat{li}", [fi, 2 * h], BF16, isOutput=False))
        attr_in.append(nc.declare_dram_parameter(f"attr{li}", [128, h], FP32, isOutput=False))
        bout_in.append(nc.declare_dram_parameter(f"bout{li}", [128, h], FP32, isOutput=False))
        bxr_in.append(nc.declare_dram_parameter(f"bxr{li}", [128, 2 * h], FP32, isOutput=False))
        gam_in.append(nc.declare_dram_parameter(f"gam{li}", [1, h], FP32, isOutput=False))
        bet_in.append(nc.declare_dram_parameter(f"bet{li}", [1, h], FP32, isOutput=False))
    pool_out = nc.declare_dram_parameter("pool", [G, 17], FP32, isOutput=True)
    if debug:
        xl_dump = nc.declare_dram_parameter("xl_dump", [PADSH, TW], FP32, isOutput=True)
        xr_dump = nc.declare_dram_parameter("xr_dump", [PADSH, TW], FP32, isOutput=True)
        accm_dump = nc.declare_dram_parameter("accm_dump", [PADSH, ACCW], FP32, isOutput=True)
        stpre_dump = nc.declare_dram_parameter("stpre_dump", [1, 128], FP32, isOutput=True)
        stpost_dump = nc.declare_dram_parameter("stpost_dump", [1, 128], FP32, isOutput=True)
        out_dump = nc.declare_dram_parameter("out_dump", [128, NT * TW], FP32, isOutput=True)

    xl_bounce = nc.dram_tensor("xl_bounce", [PADSH, TW], FP32)
    xl_full = nc.dram_tensor("xl_full", [TABN, TW], FP32, addr_space="Shared")
    xr_tab = nc.dram_tensor("xr_tab", [PADSH, TW], FP32)
    accs = [nc.dram_tensor(f"acc{r}", [PADSH, ACCW], FP32) for r in range(RACC)]
    bn_in = nc.dram_tensor("bn_in", [1, 128], FP32)
    bn_out = nc.dram_tensor("bn_out", [1, 128], FP32, addr_space="Shared")

    # block row counts in the xl table
    blk_rows = [min(BLOCK, TABN - b * BLOCK) for b in range(NBLK)]

    with tile.TileContext(nc) as tc:
        with (
            tc.tile_pool(name="persist", bufs=1) as pp,
            tc.tile_pool(name="gemm", bufs=4) as gp,
            tc.tile_pool(name="gpsum", bufs=2, space="PSUM") as gpp,
            tc.tile_pool(name="spsum", bufs=1, space="PSUM") as spp,
            tc.tile_pool(name="win", bufs=3) as wp,
            tc.tile_pool(name="ep", bufs=3) as ep,
            tc.tile_pool(name="misc", bufs=2) as mp,
        ):
            # ---- persistent loads
            xT_sb = pp.tile([128, PADSH], BF16)
            nc.sync.dma_start(out=xT_sb[:], in_=xT[:])
            ident_sb = pp.tile([128, 128], BF16)
            nc.sync.dma_start(out=ident_sb[:], in_=ident_in[:])
            iota_sb = pp.tile([128, G], FP32)
            nc.sync.dma_start(out=iota_sb[:], in_=iota_in[:])
            batch_sb = pp.tile([128, NT], FP32)
            nc.sync.dma_start(out=batch_sb[:], in_=batch_in[:])
            padmask_sb = pp.tile([128, 1], FP32)
            nc.sync.dma_start(out=padmask_sb[:], in_=padmask_in[:])
            ones_sb = pp.tile([128, 1], FP32)
            nc.vector.memset(ones_sb[:], 1.0)
            ones_row = pp.tile([1, 128], FP32)
            nc.vector.memset(ones_row[:], 1.0)
            zero_sb = pp.tile([128, 1024], FP32)
            nc.vector.memset(zero_sb[:], 0.0)
            z3 = zero_sb[:].rearrange("p (a b) -> p a b", b=128)

            for k in range(8):
                nc.sync.dma_start(out=sidx_rep[16 * k:16 * (k + 1), :], in_=sidx_in[:])
                nc.sync.dma_start(out=didx_rep[16 * k:16 * (k + 1), :], in_=didx_in[:])

            hT_sb = pp.tile([128, PADSH], BF16)   # transposed h for next layer
            out_sb = pp.tile([128, NT * TW], FP32)  # pre-BN aggregates
            hpool_sb = pp.tile([128, NT * 17], FP32)  # L3 pooling payload

            for li, (fi, h) in enumerate(LAYERS[:nlayers]):
                h2 = 2 * h
                EL = h + 1  # scatter element: [w*xl | w]
                # ---- layer consts
                wcat_sb = mp.tile([fi, h2], BF16, tag="wcat")
                nc.sync.dma_start(out=wcat_sb[:], in_=wcat_in[li][:])
                attr_sb = mp.tile([128, h], FP32, tag="attr")
                nc.sync.dma_start(out=attr_sb[:], in_=attr_in[li][:])
                bout_sb = mp.tile([128, h], FP32, tag="bout")
                nc.sync.dma_start(out=bout_sb[:], in_=bout_in[li][:])
                bxr_sb = mp.tile([128, h2], FP32, tag="bxr")
                nc.sync.dma_start(out=bxr_sb[:], in_=bxr_in[li][:])
                gam_sb = mp.tile([1, h], FP32, tag="gam")
                nc.sync.dma_start(out=gam_sb[:], in_=gam_in[li][:])
                bet_sb = mp.tile([1, h], FP32, tag="bet")
                nc.sync.dma_start(out=bet_sb[:], in_=bet_in[li][:])

                # ---- A: GEMM -> xl_bounce + xr_tab
                for t in range(NT):
                    if li == 0:
                        lhsT = xT_sb[:, t * 128:(t + 1) * 128]
                    else:
                        lhsT = hT_sb[:fi, t * 128:(t + 1) * 128]
                    ps = gpp.tile([128, h2], FP32, tag="gemm_ps")
                    nc.tensor.matmul(ps[:], lhsT, wcat_sb[:], start=True, stop=True)
                    sb = gp.tile([128, h2], FP32, tag="gemm_sb")
                    nc.vector.tensor_tensor(sb[:], ps[:], bxr_sb[:], mybir.AluOpType.add)
                    nc.sync.dma_start(
                        out=xl_bounce[t * 128:(t + 1) * 128, 0:h], in_=sb[:, 0:h])
                    nc.sync.dma_start(
                        out=xr_tab[t * 128:(t + 1) * 128, 0:h], in_=sb[:, h:h2])

                # ---- B: AllGather xl table
                nc.gpsimd.collective_compute(
                    "AllGather", mybir.AluOpType.bypass,
                    replica_groups=[list(range(NCORES))],
                    ins=[xl_bounce[:].opt()], outs=[xl_full[:].opt()],
                )

                if debug and li == 0:
                    for t in range(NT):
                        dt_ = gp.tile([128, TW], FP32, tag="dmp")
                        nc.sync.dma_start(out=dt_[:], in_=xl_bounce[t * 128:(t + 1) * 128, :])
                        nc.sync.dma_start(out=xl_dump[t * 128:(t + 1) * 128, :], in_=dt_[:])
                        dt2 = gp.tile([128, TW], FP32, tag="dmp2")
                        nc.sync.dma_start(out=dt2[:], in_=xr_tab[t * 128:(t + 1) * 128, :])
                        nc.sync.dma_start(out=xr_dump[t * 128:(t + 1) * 128, :], in_=dt2[:])

                # ---- C: zero accumulators
                for r in range(RACC):
                    for k in range(PADSH // 896):
                        acc3 = accs[r][k * 896:(k + 1) * 896, :].rearrange(
                            "(p a) b -> p a b", p=128)
                        nc.sync.dma_start(out=acc3, in_=z3[:, 0:7, :])

                # ---- D: edge windows
                for w, b in enumerate(win_bucket):
                    c0 = w * (WCAP // 16)
                    si = wp.tile([128, WCAP // 16], I16, tag="sidx")
                    nc.sync.dma_start(out=si[:], in_=sidx_rep[:, c0:c0 + WCAP // 16])
                    di = wp.tile([128, WCAP // 16], I16, tag="didx")
                    nc.sync.dma_start(out=di[:], in_=didx_rep[:, c0:c0 + WCAP // 16])
                    xlg = wp.tile([128, WT * TW], FP32, tag="xlg")
                    xl3 = xlg[:].rearrange("p (a b) -> p a b", b=TW)
                    nc.gpsimd.dma_gather(
                        out_ap=xl3,
                        in_ap=xl_full[b * BLOCK:b * BLOCK + blk_rows[b], :],
                        idxs_ap=si[:], num_idxs=WCAP, num_idxs_reg=WCAP,
                        elem_size=TW, single_packet=False,
                    )
                    xrg = wp.tile([128, WT * TW], FP32, tag="xrg")
                    xr3 = xrg[:].rearrange("p (a b) -> p a b", b=TW)
                    nc.gpsimd.dma_gather(
                        out_ap=xr3, in_ap=xr_tab[:],
                        idxs_ap=di[:], num_idxs=WCAP, num_idxs_reg=WCAP,
                        elem_size=TW, single_packet=False,
                    )
                    # s = xl + xr ; lr = lrelu(s) ; t = lr*att (all into xrg)
                    nc.vector.tensor_tensor(
                        xr3[:, :, 0:h], xl3[:, :, 0:h], xr3[:, :, 0:h],
                        mybir.AluOpType.add)
                    sc = wp.tile([128, WT * TW], FP32, tag="sc")
                    s3 = sc[:].rearrange("p (a b) -> p a b", b=TW)
                    nc.scalar.activation(
                        s3[:, :, 0:h], xr3[:, :, 0:h],
                        mybir.ActivationFunctionType.Copy, scale=NEG_SLOPE)
                    nc.vector.tensor_tensor(
                        xr3[:, :, 0:h], xr3[:, :, 0:h], s3[:, :, 0:h],
                        mybir.AluOpType.max)
                    nc.vector.tensor_tensor(
                        xr3[:, :, 0:h], xr3[:, :, 0:h],
                        attr_sb[:].unsqueeze(1).broadcast_to((128, WT, h)),
                        mybir.AluOpType.mult)
                    ew = wp.tile([128, WT], FP32, tag="ew")
                    nc.vector.tensor_reduce(
                        ew[:], xr3[:, :, 0:h], axis=mybir.AxisListType.X,
                        op=mybir.AluOpType.add)
                    nc.scalar.activation(
                        ew[:], ew[:], mybir.ActivationFunctionType.Exp)
                    pay = wp.tile([128, WT * EL], FP32, tag="pay")
                    p3 = pay[:].rearrange("p (a b) -> p a b", b=EL)
                    nc.vector.tensor_tensor(
                        p3[:, :, 0:h], xl3[:, :, 0:h],
                        ew[:].unsqueeze(2).broadcast_to((128, WT, h)),
                        mybir.AluOpType.mult)
                    nc.vector.tensor_copy(p3[:, :, h:EL], ew[:].unsqueeze(2))
                    nc.gpsimd.dma_scatter_add(
                        accs[w % RACC][:, 0:EL], p3, di[:], WCAP, WCAP, EL,
                        elem_step=ACCW, single_packet=False,
                    )

                # ---- E: merge accs, divide, bias, stats
                st2 = ep.tile([128, 2 * h], FP32, tag="st2")
                nc.vector.memset(st2[:], 0.0)
                ssum = st2[:, 0:h]
                sqsum = st2[:, h:2 * h]
                for t in range(NT):
                    at = ep.tile([128, ACCW], FP32, tag="at")
                    nc.sync.dma_start(out=at[:], in_=accs[0][t * 128:(t + 1) * 128, :])
                    for r in range(1, RACC):
                        ar = ep.tile([128, ACCW], FP32, tag="ar")
                        nc.sync.dma_start(out=ar[:], in_=accs[r][t * 128:(t + 1) * 128, :])
                        nc.vector.tensor_tensor(at[:, 0:EL], at[:, 0:EL], ar[:, 0:EL],
                                                mybir.AluOpType.add)
                    # denom -> reciprocal
                    if debug and li == 0:
                        nc.sync.dma_start(out=accm_dump[t * 128:(t + 1) * 128, 0:EL],
                                          in_=at[:, 0:EL])
                    rec = ep.tile([128, 1], FP32, tag="rec")
                    nc.vector.tensor_scalar_add(rec[:], at[:, h:EL], 1e-30)
                    nc.vector.reciprocal(rec[:], rec[:])
                    op = out_sb[:, t * TW:t * TW + h]
                    nc.vector.tensor_scalar_mul(op, at[:, 0:h], rec[:])
                    nc.vector.tensor_tensor(op, op, bout_sb[:], mybir.AluOpType.add)
                    if t == NT - 1:
                        # zero pad nodes 12500..12543 (partitions 84.. of the
                        # last tile) via per-partition mask multiply
                        nc.vector.tensor_scalar_mul(op, op, padmask_sb[:])
                    nc.vector.tensor_tensor(ssum, ssum, op, mybir.AluOpType.add)
                    sq = ep.tile([128, h], FP32, tag="sq")
                    nc.scalar.activation(sq[:], op, mybir.ActivationFunctionType.Square)
                    nc.vector.tensor_tensor(sqsum, sqsum, sq[:], mybir.AluOpType.add)

                # partition-reduce stats via ones matmul -> [1, 2h]
                stp = spp.tile([1, 2 * h], FP32, tag="stats_ps")
                nc.tensor.matmul(stp[:], ones_sb[:, 0:1], st2[:], start=True, stop=True)
                st_sb = ep.tile([1, 128], FP32, tag="st_sb")
                nc.vector.memset(st_sb[:], 0.0)
                nc.scalar.copy(st_sb[0:1, 0:2 * h], stp[:])
                nc.sync.dma_start(out=bn_in[:], in_=st_sb[:])
                if debug and li == 0:
                    nc.sync.dma_start(out=stpre_dump[:], in_=st_sb[:])
                nc.gpsimd.collective_compute(
                    "AllReduce", mybir.AluOpType.add,
                    replica_groups=[list(range(NCORES))],
                    ins=[bn_in[:].opt()], outs=[bn_out[:].opt()],
                )
                stg = ep.tile([1, 128], FP32, tag="stg")
                nc.sync.dma_start(out=stg[:], in_=bn_out[:])
                if debug and li == 0:
                    nc.sync.dma_start(out=stpost_dump[:], in_=stg[:])
                # a = gamma * rsqrt(var+eps); b = beta - mu * a   (on 1 partition)
                mu = ep.tile([1, h], FP32, tag="mu")
                nc.vector.tensor_scalar_mul(mu[:], stg[0:1, 0:h], 1.0 / N0)
                var = ep.tile([1, h], FP32, tag="var")
                nc.vector.tensor_scalar_mul(var[:], stg[0:1, h:2 * h], 1.0 / N0)
                musq = ep.tile([1, h], FP32, tag="musq")
                nc.scalar.activation(musq[:], mu[:], mybir.ActivationFunctionType.Square)
                nc.vector.tensor_tensor(var[:], var[:], musq[:], mybir.AluOpType.subtract)
                nc.vector.tensor_scalar_add(var[:], var[:], EPS)
                nc.scalar.activation(var[:], var[:], mybir.ActivationFunctionType.Sqrt)
                nc.vector.reciprocal(var[:], var[:])   # rsqrt
                arow = ep.tile([1, h], FP32, tag="arow")
                nc.vector.tensor_tensor(arow[:], var[:], gam_sb[:], mybir.AluOpType.mult)
                brow = ep.tile([1, h], FP32, tag="brow")
                nc.vector.tensor_tensor(brow[:], mu[:], arow[:], mybir.AluOpType.mult)
                nc.vector.tensor_tensor(brow[:], bet_sb[:], brow[:], mybir.AluOpType.subtract)
                # broadcast a,b across partitions via K=1 matmul
                abp = spp.tile([128, h], FP32, tag="ab_ps")
                nc.tensor.matmul(abp[:], ones_row[:], arow[:], start=True, stop=True)
                a_bc = ep.tile([128, h], FP32, tag="a_bc")
                nc.scalar.copy(a_bc[:], abp[:])
                abp2 = spp.tile([128, h], FP32, tag="ab_ps")
                nc.tensor.matmul(abp2[:], ones_row[:], brow[:], start=True, stop=True)
                b_bc = ep.tile([128, h], FP32, tag="b_bc")
                nc.scalar.copy(b_bc[:], abp2[:])

                # ---- normalize + relu (+ transpose or pooling payload)
                if li < 2:
                    for t in range(NT):
                        op = out_sb[:, t * TW:t * TW + h]
                        hb = gp.tile([128, h], BF16, tag="hb")
                        nc.vector.tensor_tensor(op, op, a_bc[:], mybir.AluOpType.mult)
                        nc.vector.tensor_tensor(op, op, b_bc[:], mybir.AluOpType.add)
                        nc.scalar.activation(hb[:], op,
                                             mybir.ActivationFunctionType.Relu)
                        tp = gpp.tile([h, 128], BF16, tag="tr_ps")
                        nc.tensor.transpose(tp[:], hb[:], ident_sb[:])
                        nc.scalar.copy(hT_sb[0:h, t * 128:(t + 1) * 128], tp[:])
                else:
                    for t in range(NT):
                        op = out_sb[:, t * TW:t * TW + h]
                        nc.vector.tensor_tensor(op, op, a_bc[:], mybir.AluOpType.mult)
                        nc.vector.tensor_tensor(op, op, b_bc[:], mybir.AluOpType.add)
                        hp3 = hpool_sb[:, t * 17:(t + 1) * 17]
                        nc.scalar.activation(hp3[0:128, 0:h], op,
                                             mybir.ActivationFunctionType.Relu)
                        nc.vector.memset(hpool_sb[:, t * 17 + 16:t * 17 + 17], 1.0)

            if debug:
                nc.sync.dma_start(out=out_dump[:], in_=out_sb[:])

            # ---- pooling: indicator matmuls accumulate [G, 17]
            if nlayers == 3:
                plp = spp.tile([G, 17], FP32, tag="pool_ps")
                for t in range(NT):
                    ind = gp.tile([128, G], FP32, tag="ind")
                    nc.vector.tensor_tensor(
                        ind[:], batch_sb[:, t:t + 1].broadcast_to((128, G)),
                        iota_sb[:], mybir.AluOpType.is_equal)
                    nc.tensor.matmul(plp[:], ind[:], hpool_sb[:, t * 17:(t + 1) * 17],
                                     start=(t == 0), stop=(t == NT - 1))
                pl_sb = gp.tile([G, 17], FP32, tag="pl_sb")
                nc.scalar.copy(pl_sb[:], plp[:])
                nc.sync.dma_start(out=pool_out[:], in_=pl_sb[:])
            else:
                nc.vector.memset(hpool_sb[:], 0.0)
                pl_sb = gp.tile([G, 17], FP32, tag="pl_sb")
                nc.vector.memset(pl_sb[:], 0.0)
                nc.sync.dma_start(out=pool_out[:], in_=pl_sb[:])

    nc.compile()
    return nc


# ---------------------------------------------------------------- kernel()

LAST_EXEC_NS = None

def _make_in_maps(inputs):
    x = np.asarray(inputs["x"], np.float32)
    ei = np.asarray(inputs["edge_index"], np.int64)
    batch = np.asarray(inputs["batch"], np.int64)
    src_w, dst_w, win_bucket = _build_windows(ei)
    # xT shards [128, PADSH] bf16
    xT = np.zeros((NCORES, 128, PADSH), ml_dtypes.bfloat16)
    xt_full = np.ascontiguousarray(x.T).astype(ml_dtypes.bfloat16)  # [128, N]
    for c in range(NCORES):
        xT[c, :, :SHARD] = xt_full[:, c * SHARD:(c + 1) * SHARD]

    ident = np.eye(128, dtype=ml_dtypes.bfloat16)
    iota = np.tile(np.arange(G, dtype=np.float32)[None, :], (128, 1))
    batchf = np.full((NCORES, 128, NT), -1.0, np.float32)
    for c in range(NCORES):
        loc = batch[c * SHARD:(c + 1) * SHARD].astype(np.float32)
        pad = np.full(PADSH - SHARD, -1.0, np.float32)
        batchf[c] = np.concatenate([loc, pad]).reshape(NT, 128).T

    const_maps = {"ident": ident, "iota": iota}
    for li, (fi, h) in enumerate(LAYERS):
        Wl = np.asarray(inputs[f"Wl{li+1}"], np.float32)
        bl = np.asarray(inputs[f"bl{li+1}"], np.float32)
        Wr = np.asarray(inputs[f"Wr{li+1}"], np.float32)
        br = np.asarray(inputs[f"br{li+1}"], np.float32)
        att = np.asarray(inputs[f"att{li+1}"], np.float32)
        bias = np.asarray(inputs[f"bias{li+1}"], np.float32)
        gam = np.asarray(inputs[f"gamma{li+1}"], np.float32)
        bet = np.asarray(inputs[f"beta{li+1}"], np.float32)
        const_maps[f"wcat{li}"] = np.concatenate([Wl, Wr], 1).astype(ml_dtypes.bfloat16)
        const_maps[f"attr{li}"] = np.tile(att[None, :], (128, 1)).astype(np.float32)
        const_maps[f"bout{li}"] = np.tile((bl + bias)[None, :], (128, 1)).astype(np.float32)
        bxr = np.concatenate([np.zeros(h, np.float32), bl + br])
        const_maps[f"bxr{li}"] = np.tile(bxr[None, :], (128, 1)).astype(np.float32)
        const_maps[f"gam{li}"] = gam[None, :].astype(np.float32)
        const_maps[f"bet{li}"] = bet[None, :].astype(np.float32)

    padmask = (np.arange(128) + (NT - 1) * 128 < SHARD).astype(np.float32)[:, None]
    in_maps = [
        {"xT": xT[c], "sidx": src_w[c], "didx": dst_w[c], "batchf": batchf[c],
         "padmask": padmask, **const_maps}
        for c in range(NCORES)
    ]
    return in_maps, win_bucket


def kernel(**inputs):
    in_maps, win_bucket = _make_in_maps(inputs)
    nc = _build_nc(win_bucket)

    global LAST_EXEC_NS
    t0 = time.time()
    res = run_bass_kernel_spmd(nc, in_maps, core_ids=list(range(NCORES)))
    LAST_EXEC_NS = (time.time() - t0) * 1e9

    pool = np.zeros((G, 17), np.float64)
    for c in range(NCORES):
        pool += np.asarray(res.results[c]["pool"], np.float64)
    sums, cnt = pool[:, :16], pool[:, 16]
    pooled = sums / np.maximum(cnt, 1.0)[:, None]
    linW = np.asarray(inputs["linW"], np.float32)
    linb = np.asarray(inputs["linb"], np.float32)
    return (pooled.astype(np.float32) @ linW + linb).astype(np.float32)
